# revision 27
# baseline (speedup 1.0000x reference)
"""NT-Xent loss kernel for Trainium2 (8 NeuronCores, SPMD) — symmetric v2.

Math (derived from the reference):
  z = concat(z_i, z_j)                         (N=8192, D=128)
  zn = z / max(||z||, 1e-8)
  K[a,b] = zn[a].zn[b]; G = K/temp = 2K
  S_a = sum_b e^{G[a,b]}   (over ALL b, incl. a)
  loss = mean_a [ ln(S_a - e^{G[a,p(a)]}) - G[a,a] ],  p(a) = (a+4096)%8192

Sharding + symmetry: core c's input is z np.roll'd by -1024c rows, so its
local rows [0,1024) are its global row block.  In local tile coords
(64 tiles of 128 rows), core c computes tiles (r', t') for r' in [0,8)
and t' in [r', r'+32] — tile offsets d = t'-r' in [0,32].  Globally each
unordered off-diag pair appears with offsets d and 64-d; exactly one of
them is <= 31, so offsets 1..31 cover each pair once.  Offset-32 tiles
(positive-pair blocks) are computed by BOTH involved cores, row-sum only;
offset 0 (diagonal) by its owner, row-sum only.  Row sums come from ACT
Exp accum_out over the computed strip; the mirrored contributions (what
offsets 33..63 would have provided) come from COLUMN sums of the
offset-1..31 tiles, computed on the PE as ones^T @ e_piece accumulated
into a persistent 1-bank PSUM [10,512] indexed by local column.  The
host adds row partials + column partials across cores into S[8192],
assembles the pos/diag extracts, and finishes with ln over 8192 rows.

Because only local columns [0, 5120) are ever referenced, each core's
input is just rows [0,5120) of its rolled z: 2.5 MiB DMA instead of 4.

Per-core device pipeline:
  1. DMA z [5120,128] f32 as 5 chunks of 8 natural tiles [128,128].
  2. n2 via fused square-accum stt per tile (DVE/GpSimd split);
     r = exp(-0.5*ln(max(n2,1e-16))) on ACT (same table set as Exp).
  3. zn_bf16 = z * r (DVE broadcast mult, f32 -> bf16); PE-transpose each
     chunk's 8 tiles into one PSUM bank (bf16); DVE-copy to znT.
  4. Per block-row r': strip of 33 tiles as PSUM chunks [1536,1536,1152];
     PE matmul (bf16) -> ACT Exp(scale=2, accum_out=row partial, out=e_sb
     bf16) -> PE ones-matmul column-sum pieces into cs PSUM (delayed one
     strip so PE's G matmuls stay ahead of ACT).  DVE extracts diag
     (strip col 0) and pos (strip col 4096) via identity-masked accum.
  5. Outputs: misc[128,24] = (row sums | diag2 | pos2), cs[10,512].
"""

import numpy as np

N = 8192
D = 128
NCORES = 8
RPC = N // NCORES  # 1024 rows per core
NTL = 40  # local tiles referenced (cols [0, 5120))
LROWS = NTL * 128  # 5120
RT = 8  # block-rows per core
STRIP_W = 33 * 128  # 4224
CHUNKS = (1536, 1536, 1152)  # strips 1-7
CHUNKS0 = (512, 1024, 1536, 1152)  # strip 0: small lead chunks start ACT early
NCH = 4  # max chunks per strip (accs slots)
DMA_TILES = (4, 4, 8, 8, 8, 8)  # input chunk sizes (tiles)
DMA_CH = len(DMA_TILES)

_NC_CACHE = {}


def _chunks(rp):
    return CHUNKS0 if rp == 0 else CHUNKS


def _chunk_starts(rp):
    starts, acc = [], 0
    for w in _chunks(rp):
        starts.append(acc)
        acc += w
    return starts


def _cs_pieces(rp):
    """Column-sum pieces for block-row rp: local cols
    [(rp+1)*128, (rp+32)*128), split at 512 boundaries (cs PSUM banks) and
    at strip-chunk boundaries (e_sb source tiles).  Returns
    (c0, w, k, eoff): absolute col, width, chunk index, offset in chunk."""
    lo = (rp + 1) * 128
    hi = (rp + 32) * 128
    base = rp * 128
    starts = _chunk_starts(rp)
    bounds = [base + s for s in starts] + [base + STRIP_W]
    pieces = []
    c = lo
    while c < hi:
        nxt = min((c // 512 + 1) * 512, hi)
        for b in bounds:
            if c < b < nxt:
                nxt = b
        k = max(i for i, b in enumerate(bounds[:-1]) if b <= c)
        pieces.append((c, nxt - c, k, c - bounds[k]))
        c = nxt
    return pieces


_N_PIECES = sum(len(_cs_pieces(rp)) for rp in range(RT))

# main-loop work order: (rp, k) sorted by the last znT column each chunk
# needs, so ACT never waits on a late input chunk
_WORK = sorted(
    ((rp, k) for rp in range(RT) for k in range(len(_chunks(rp)))),
    key=lambda w: (w[0] * 128 + _chunk_starts(w[0])[w[1]] + _chunks(w[0])[w[1]], w[0]),
)


def _build_nc(reps: int = 1):
    from contextlib import ExitStack

    import concourse.bass as bass  # noqa: F401
    import concourse.tile as tile
    from concourse import bacc, mybir
    from concourse.masks import make_identity

    f32 = mybir.dt.float32
    bf16 = mybir.dt.bfloat16
    FN = mybir.ActivationFunctionType
    ALU = mybir.AluOpType

    nc = bacc.Bacc(
        "TRN2", target_bir_lowering=False, debug=False, num_devices=NCORES
    )
    z_nat = nc.dram_tensor("z_nat", [LROWS, D], f32, kind="ExternalInput")
    misc_out = nc.dram_tensor("misc", [128, 3 * RT], f32, kind="ExternalOutput")
    cs_out = nc.dram_tensor("cs", [10, 512], f32, kind="ExternalOutput")

    def body(pools, consts):
        big, small, scr, epool, psg, pscs, pstr = pools
        ident_bf, onehot_bf, zeros_bf = consts

        # ---- input DMA: variable chunks (small lead chunks), f32 ----
        z_sb = big.tile([128, NTL, 128], f32, tag="z_nat")
        z_nat_t = z_nat.ap().rearrange("(t p) d -> p t d", p=128)
        t0 = 0
        dma_slices = []
        for ntiles in DMA_TILES:
            sl = slice(t0, t0 + ntiles)
            dma_slices.append(sl)
            nc.sync.dma_start(z_sb[:, sl, :], z_nat_t[:, sl, :])
            t0 += ntiles

        n2 = small.tile([128, NTL], f32, tag="n2")
        nmax = small.tile([128, NTL], f32, tag="nmax")
        r_nat = small.tile([128, NTL], f32, tag="r_nat")
        zn_bf = big.tile([128, NTL, 128], bf16, tag="zn_bf")
        znt = big.tile([128, LROWS], bf16, tag="znt")
        sq_scr = scr.tile([128, 128], f32, tag="sq_scr")
        sq_scr2 = scr.tile([128, 128], f32, tag="sq_scr2")
        m_scr = scr.tile([128, 128], bf16, tag="m_scr")
        accs = small.tile([128, RT, NCH], f32, tag="accs")
        misc_sb = small.tile([128, 3 * RT], f32, tag="misc_sb")
        cs_sb = small.tile([10, 512], f32, tag="cs_sb")

        # ---- PE warmup: dummy transposes while the first DMA lands, so
        # the HAM clock gate is released before real work arrives ----
        ps_w = pstr.tile([128, 512], f32, tag="ps_tr")
        ps_wb = ps_w[:].bitcast(bf16)
        for i in range(32):
            nc.tensor.transpose(
                ps_wb[:, (i % 8) * 128 : (i % 8 + 1) * 128],
                ident_bf[:],
                ident_bf[:],
            )

        # memset accs once (strips 1-7 leave slot 3 unwritten)
        nc.gpsimd.memset(accs[:], 0.0)

        # ---- prefix, per input chunk: n2 -> r -> zn_bf -> transpose ----
        for q, sl in enumerate(dma_slices):
            ntiles = sl.stop - sl.start
            for t in range(sl.start, sl.stop):
                src = z_sb[:, t, :]
                scrt = sq_scr if (t % 2) == 0 else sq_scr2
                nc.vector.scalar_tensor_tensor(
                    out=scrt[:],
                    in0=src,
                    scalar=1.0,
                    in1=src,
                    op0=ALU.mult,
                    op1=ALU.mult,
                    accum_out=n2[:, t : t + 1],
                )
            nc.vector.tensor_scalar_max(
                out=nmax[:, sl], in0=n2[:, sl], scalar1=1e-16
            )
            lnn = scr.tile([128, 8], f32, tag=f"lnn{q % 2}")
            nc.scalar.activation(
                out=lnn[:, 0:ntiles], in_=nmax[:, sl], func=FN.Ln
            )
            nc.scalar.activation(
                out=r_nat[:, sl], in_=lnn[:, 0:ntiles], func=FN.Exp, scale=-0.5
            )
            # zn_bf = z * r  (broadcast r along d, f32 -> bf16)
            r_b = r_nat[:, sl].unsqueeze(2).broadcast_to((128, ntiles, 128))
            nc.vector.scalar_tensor_tensor(
                out=zn_bf[:, sl, :],
                in0=z_sb[:, sl, :],
                scalar=1.0,
                in1=r_b,
                op0=ALU.mult,
                op1=ALU.mult,
            )
            # transpose the chunk's tiles into one PSUM bank (bf16)
            ps_t = pstr.tile([128, 512], f32, tag="ps_tr")
            ps_tb = ps_t[:].bitcast(bf16)
            for i, t in enumerate(range(sl.start, sl.stop)):
                nc.tensor.transpose(
                    ps_tb[:, i * 128 : (i + 1) * 128],
                    zn_bf[:, t, :],
                    ident_bf[:],
                )
            nc.vector.tensor_copy(
                out=znt[:, sl.start * 128 : sl.stop * 128],
                in_=ps_tb[:, 0 : ntiles * 128],
            )

        # ---- zero-pass for cs PSUM (sets has_written on all elements).
        # cs PSUM is [32, 512]: row q accumulates local cols [512q, 512q+512)
        # via one-hot stationary operands (PE outputs must start at
        # partition 0/32/64, so rows are selected by the one-hot instead).
        cs_ps = pscs.tile([32, 512], f32, tag="cs_ps")
        nc.tensor.matmul(
            cs_ps[:, :],
            zeros_bf[:],
            znt[:, 0:512],
            start=True,
            stop=False,
            skip_group_check=True,
        )

        # ---- main: work-list order (sorted by last znT column needed) ----
        piece_ctr = [0]
        emitted = {}

        def emit_cs(rp, e_list, upto):
            for c0, w, k, eoff in _cs_pieces(rp):
                if c0 < emitted.get(rp, 0):
                    continue
                if c0 + w > rp * 128 + upto:
                    break
                emitted[rp] = c0 + w
                qq = c0 // 512
                piece_ctr[0] += 1
                nc.tensor.matmul(
                    cs_ps[:, c0 - qq * 512 : c0 - qq * 512 + w],
                    onehot_bf[:, 10 - qq : 42 - qq],
                    e_list[k][:, eoff : eoff + w],
                    start=False,
                    stop=(piece_ctr[0] == _N_PIECES),
                    skip_group_check=True,
                )

        e_tiles = [[None] * len(_chunks(rp)) for rp in range(RT)]
        for rp, k in _WORK:
            lhsT = znt[:, rp * 128 : (rp + 1) * 128]
            base = rp * 128
            w_k = _chunks(rp)[k]
            s_k = _chunk_starts(rp)[k]
            last_k = k == len(_chunks(rp)) - 1
            G_ps = psg.tile([128, 1536], f32, tag="ps_g")
            c0 = base + s_k
            for off in range(0, w_k, 512):
                w = min(512, w_k - off)
                nc.tensor.matmul(
                    G_ps[:, off : off + w],
                    lhsT,
                    znt[:, c0 + off : c0 + off + w],
                    start=True,
                    stop=True,
                )
            e_sb = epool.tile([128, 1536], bf16, tag="e_sb")
            e_tiles[rp][k] = e_sb
            nc.scalar.activation(
                out=e_sb[:, 0:w_k],
                in_=G_ps[:, 0:w_k],
                func=FN.Exp,
                scale=2.0,
                accum_out=accs[:, rp, k : k + 1],
            )
            # diag/pos come from e_sb (SBUF, bf16): masked accum picks
            # e^{2K[a,a]} / e^{2K[a,p(a)]}; host takes log.  Reading e_sb
            # instead of G_ps keeps DVE off the PSUM critical path.
            if k == 0:
                nc.vector.scalar_tensor_tensor(
                    out=m_scr[:],
                    in0=e_sb[:, 0:128],
                    scalar=1.0,
                    in1=ident_bf[:],
                    op0=ALU.mult,
                    op1=ALU.mult,
                    accum_out=misc_sb[:, RT + rp : RT + rp + 1],
                )
            if last_k:
                nc.vector.scalar_tensor_tensor(
                    out=m_scr[:],
                    in0=e_sb[:, 1024:1152],
                    scalar=1.0,
                    in1=ident_bf[:],
                    op0=ALU.mult,
                    op1=ALU.mult,
                    accum_out=misc_sb[:, 2 * RT + rp : 2 * RT + rp + 1],
                )
            emit_cs(rp, e_tiles[rp], s_k + w_k)

        # ---- epilogue ----
        nc.vector.reduce_sum(
            out=misc_sb[:, 0:RT], in_=accs[:], axis=mybir.AxisListType.X
        )
        nc.vector.tensor_copy(out=cs_sb[:], in_=cs_ps[0:10, :])
        nc.sync.dma_start(misc_out.ap()[:, :], misc_sb[:])
        nc.scalar.dma_start(cs_out.ap()[:, :], cs_sb[:])

    # Pin every ACT function to the one table set containing Ln/Exp so the
    # table-load pass never inserts mid-kernel switches (~2.7us each).
    import concourse.hw_specs as hw_specs

    _real_tables = hw_specs.get_activation_tables(nc.m.arch)
    _pruned = {
        name: (fns if name == "natural_log_exp_and_others" else set())
        for name, fns in _real_tables.items()
    }
    _orig_get_tables = bacc.get_activation_tables

    with tile.TileContext(nc) as tc, ExitStack() as ctx:
        const_pool = ctx.enter_context(tc.tile_pool(name="const", bufs=1))
        big = ctx.enter_context(tc.tile_pool(name="big", bufs=1))
        small = ctx.enter_context(tc.tile_pool(name="small", bufs=1))
        scr = ctx.enter_context(tc.tile_pool(name="scr", bufs=1))
        epool = ctx.enter_context(tc.tile_pool(name="epool", bufs=28))
        psg = ctx.enter_context(tc.tile_pool(name="psg", bufs=2, space="PSUM"))
        pscs = ctx.enter_context(tc.tile_pool(name="pscs", bufs=1, space="PSUM"))
        pstr = ctx.enter_context(tc.tile_pool(name="pstr", bufs=1, space="PSUM"))

        import concourse.mybir as mybir

        ident_bf = const_pool.tile([128, 128], mybir.dt.bfloat16, tag="idbf")
        make_identity(nc, ident_bf[:])
        # onehot_bf[:, 10] == 1, else 0; slices [10-q : 42-q] give [128, 32]
        # one-hot stationaries selecting cs PSUM row q.
        onehot_bf = const_pool.tile([128, 42], mybir.dt.bfloat16, tag="onehot")
        nc.vector.memset(onehot_bf[:], 0.0)
        nc.vector.memset(onehot_bf[:, 10:11], 1.0)
        zeros_bf = const_pool.tile([128, 32], mybir.dt.bfloat16, tag="zeros")
        nc.vector.memset(zeros_bf[:], 0.0)

        for _rep in range(reps):
            body(
                (big, small, scr, epool, psg, pscs, pstr),
                (ident_bf, onehot_bf, zeros_bf),
            )

    bacc.get_activation_tables = lambda arch: _pruned
    try:
        nc.compile()
    finally:
        bacc.get_activation_tables = _orig_get_tables
    return nc


def get_nc(reps: int = 1):
    if reps not in _NC_CACHE:
        _NC_CACHE[reps] = _build_nc(reps)
    return _NC_CACHE[reps]


def make_in_maps(z_i: np.ndarray, z_j: np.ndarray):
    z = np.concatenate(
        [np.asarray(z_i, np.float32), np.asarray(z_j, np.float32)], axis=0
    )
    in_maps = []
    for c in range(NCORES):
        zr = np.roll(z, -c * RPC, axis=0)[:LROWS]
        in_maps.append({"z_nat": np.ascontiguousarray(zr)})
    return in_maps


def gather(results) -> np.ndarray:
    S = np.zeros(N, np.float64)
    diag2 = np.zeros(N, np.float64)
    pos2 = np.zeros(N, np.float64)
    for c, res in enumerate(results):
        misc = res["misc"].astype(np.float64)  # [128, 24]
        cs = res["cs"].astype(np.float64).reshape(-1)  # local cols 0..5119
        rows0 = c * RPC
        for rp in range(RT):
            rr = slice(rows0 + rp * 128, rows0 + (rp + 1) * 128)
            S[rr] += misc[:, rp]
            diag2[rr] = np.log(misc[:, RT + rp])
            pos2[rr] = np.log(misc[:, 2 * RT + rp])
        idx = (rows0 + np.arange(cs.shape[0])) % N
        np.add.at(S, idx, cs)
    loss = (np.log(S - np.exp(pos2)) - diag2).sum() / N
    return np.float32(loss)


def kernel(z_i: np.ndarray, z_j: np.ndarray, **run_kwargs) -> np.ndarray:
    from concourse.bass_utils import run_bass_kernel_spmd

    nc = get_nc()
    in_maps = make_in_maps(z_i, z_j)
    res = run_bass_kernel_spmd(
        nc, in_maps, core_ids=list(range(NCORES)), **run_kwargs
    )
    out = gather(res.results)
    kernel.last_results = res
    return out


# revision 30
# speedup vs baseline: 1.3331x; 1.3331x over previous
"""NT-Xent loss kernel for Trainium2 (8 NeuronCores, SPMD) — symmetric v2.

Math (derived from the reference):
  z = concat(z_i, z_j)                         (N=8192, D=128)
  zn = z / max(||z||, 1e-8)
  K[a,b] = zn[a].zn[b]; G = K/temp = 2K
  S_a = sum_b e^{G[a,b]}   (over ALL b, incl. a)
  loss = mean_a [ ln(S_a - e^{G[a,p(a)]}) - G[a,a] ],  p(a) = (a+4096)%8192

Sharding + symmetry: core c's input is z np.roll'd by -1024c rows, so its
local rows [0,1024) are its global row block.  In local tile coords
(64 tiles of 128 rows), core c computes tiles (r', t') for r' in [0,8)
and t' in [r', r'+32] — tile offsets d = t'-r' in [0,32].  Globally each
unordered off-diag pair appears with offsets d and 64-d; exactly one of
them is <= 31, so offsets 1..31 cover each pair once.  Offset-32 tiles
(positive-pair blocks) are computed by BOTH involved cores, row-sum only;
offset 0 (diagonal) by its owner, row-sum only.  Row sums come from ACT
Exp accum_out over the computed strip; the mirrored contributions (what
offsets 33..63 would have provided) come from COLUMN sums of the
offset-1..31 tiles, computed on the PE as ones^T @ e_piece accumulated
into a persistent 1-bank PSUM [10,512] indexed by local column.  The
host adds row partials + column partials across cores into S[8192],
assembles the pos/diag extracts, and finishes with ln over 8192 rows.

Because only local columns [0, 5120) are ever referenced, each core's
input is just rows [0,5120) of its rolled z: 2.5 MiB DMA instead of 4.

Per-core device pipeline:
  1. DMA z [5120,128] f32 as 5 chunks of 8 natural tiles [128,128].
  2. n2 via fused square-accum stt per tile (DVE/GpSimd split);
     r = exp(-0.5*ln(max(n2,1e-16))) on ACT (same table set as Exp).
  3. zn_bf16 = z * r (DVE broadcast mult, f32 -> bf16); PE-transpose each
     chunk's 8 tiles into one PSUM bank (bf16); DVE-copy to znT.
  4. Per block-row r': strip of 33 tiles as PSUM chunks [1536,1536,1152];
     PE matmul (bf16) -> ACT Exp(scale=2, accum_out=row partial, out=e_sb
     bf16) -> PE ones-matmul column-sum pieces into cs PSUM (delayed one
     strip so PE's G matmuls stay ahead of ACT).  DVE extracts diag
     (strip col 0) and pos (strip col 4096) via identity-masked accum.
  5. Outputs: misc[128,24] = (row sums | diag2 | pos2), cs[10,512].
"""

import numpy as np

N = 8192
D = 128
NCORES = 8
RPC = N // NCORES  # 1024 rows per core
NTL = 40  # local tiles referenced (cols [0, 5120))
LROWS = NTL * 128  # 5120
RT = 8  # block-rows per core
STRIP_W = 33 * 128  # 4224
CHUNKS = (1536, 1536, 1152)  # strips 1-7
CHUNKS0 = (512, 1024, 1536, 1152)  # strip 0: small lead chunks start ACT early
NCH = 4  # max chunks per strip (accs slots)
DMA_TILES = (4, 4, 8, 8, 8, 8)  # input chunk sizes (tiles)
DMA_CH = len(DMA_TILES)

_NC_CACHE = {}


def _chunks(rp):
    return CHUNKS0 if rp == 0 else CHUNKS


def _chunk_starts(rp):
    starts, acc = [], 0
    for w in _chunks(rp):
        starts.append(acc)
        acc += w
    return starts


def _cs_pieces(rp):
    """Column-sum pieces for block-row rp: local cols
    [(rp+1)*128, (rp+32)*128), split at 512 boundaries (cs PSUM banks) and
    at strip-chunk boundaries (e_sb source tiles).  Returns
    (c0, w, k, eoff): absolute col, width, chunk index, offset in chunk."""
    lo = (rp + 1) * 128
    hi = (rp + 32) * 128
    base = rp * 128
    starts = _chunk_starts(rp)
    bounds = [base + s for s in starts] + [base + STRIP_W]
    pieces = []
    c = lo
    while c < hi:
        nxt = min((c // 512 + 1) * 512, hi)
        for b in bounds:
            if c < b < nxt:
                nxt = b
        k = max(i for i, b in enumerate(bounds[:-1]) if b <= c)
        pieces.append((c, nxt - c, k, c - bounds[k]))
        c = nxt
    return pieces


_N_PIECES = sum(len(_cs_pieces(rp)) for rp in range(RT))

# main-loop work order: (rp, k) sorted by the last znT column each chunk
# needs, so ACT never waits on a late input chunk
_WORK = sorted(
    ((rp, k) for rp in range(RT) for k in range(len(_chunks(rp)))),
    key=lambda w: (w[0] * 128 + _chunk_starts(w[0])[w[1]] + _chunks(w[0])[w[1]], w[0]),
)


def _build_nc(reps: int = 1):
    from contextlib import ExitStack

    import concourse.bass as bass  # noqa: F401
    import concourse.tile as tile
    from concourse import bacc, mybir
    from concourse.masks import make_identity

    f32 = mybir.dt.float32
    bf16 = mybir.dt.bfloat16
    FN = mybir.ActivationFunctionType
    ALU = mybir.AluOpType

    nc = bacc.Bacc(
        "TRN2", target_bir_lowering=False, debug=False, num_devices=NCORES
    )
    z_nat = nc.dram_tensor("z_nat", [LROWS, D], f32, kind="ExternalInput")
    misc_out = nc.dram_tensor("misc", [128, 3 * RT], f32, kind="ExternalOutput")
    cs_out = nc.dram_tensor("cs", [10, 512], f32, kind="ExternalOutput")

    def body(pools, consts):
        big, small, scr, epool, psg, pscs, pstr = pools
        ident_bf, onehot_bf, zeros_bf = consts

        # ---- input DMA: variable chunks (small lead chunks), f32 ----
        z_sb = big.tile([128, NTL, 128], f32, tag="z_nat")
        z_nat_t = z_nat.ap().rearrange("(t p) d -> p t d", p=128)
        t0 = 0
        dma_slices = []
        for ntiles in DMA_TILES:
            sl = slice(t0, t0 + ntiles)
            dma_slices.append(sl)
            nc.sync.dma_start(z_sb[:, sl, :], z_nat_t[:, sl, :])
            t0 += ntiles

        n2 = small.tile([128, NTL], f32, tag="n2")
        nmax = small.tile([128, NTL], f32, tag="nmax")
        r_nat = small.tile([128, NTL], f32, tag="r_nat")
        zn_bf = big.tile([128, NTL, 128], bf16, tag="zn_bf")
        znt = big.tile([128, LROWS], bf16, tag="znt")
        sq_scr = scr.tile([128, 128], f32, tag="sq_scr")
        sq_scr2 = scr.tile([128, 128], f32, tag="sq_scr2")
        m_scr = scr.tile([128, 128], bf16, tag="m_scr")
        accs = small.tile([128, RT, NCH], f32, tag="accs")
        misc_sb = small.tile([128, 3 * RT], f32, tag="misc_sb")
        cs_sb = small.tile([10, 512], f32, tag="cs_sb")

        # ---- PE warmup: dummy transposes while the first DMA lands, so
        # the HAM clock gate is released before real work arrives ----
        ps_w = pstr.tile([128, 512], f32, tag="ps_tr")
        ps_wb = ps_w[:].bitcast(bf16)
        for i in range(32):
            nc.tensor.transpose(
                ps_wb[:, (i % 8) * 128 : (i % 8 + 1) * 128],
                ident_bf[:],
                ident_bf[:],
            )

        # memset accs once (strips 1-7 leave slot 3 unwritten)
        nc.gpsimd.memset(accs[:], 0.0)

        # ---- prefix, per input chunk: n2 -> r -> zn_bf -> transpose ----
        for q, sl in enumerate(dma_slices):
            ntiles = sl.stop - sl.start
            for t in range(sl.start, sl.stop):
                src = z_sb[:, t, :]
                scrt = sq_scr if (t % 2) == 0 else sq_scr2
                nc.vector.scalar_tensor_tensor(
                    out=scrt[:],
                    in0=src,
                    scalar=1.0,
                    in1=src,
                    op0=ALU.mult,
                    op1=ALU.mult,
                    accum_out=n2[:, t : t + 1],
                )
            nc.vector.tensor_scalar_max(
                out=nmax[:, sl], in0=n2[:, sl], scalar1=1e-16
            )
            lnn = scr.tile([128, 8], f32, tag=f"lnn{q % 2}")
            nc.scalar.activation(
                out=lnn[:, 0:ntiles], in_=nmax[:, sl], func=FN.Ln
            )
            nc.scalar.activation(
                out=r_nat[:, sl], in_=lnn[:, 0:ntiles], func=FN.Exp, scale=-0.5
            )
            # zn_bf = z * r  (broadcast r along d, f32 -> bf16)
            r_b = r_nat[:, sl].unsqueeze(2).broadcast_to((128, ntiles, 128))
            nc.vector.scalar_tensor_tensor(
                out=zn_bf[:, sl, :],
                in0=z_sb[:, sl, :],
                scalar=1.0,
                in1=r_b,
                op0=ALU.mult,
                op1=ALU.mult,
            )
            # transpose the chunk's tiles into one PSUM bank (bf16)
            ps_t = pstr.tile([128, 512], f32, tag="ps_tr")
            ps_tb = ps_t[:].bitcast(bf16)
            for i, t in enumerate(range(sl.start, sl.stop)):
                nc.tensor.transpose(
                    ps_tb[:, i * 128 : (i + 1) * 128],
                    zn_bf[:, t, :],
                    ident_bf[:],
                )
            nc.vector.tensor_copy(
                out=znt[:, sl.start * 128 : sl.stop * 128],
                in_=ps_tb[:, 0 : ntiles * 128],
            )

        # ---- zero-pass for cs PSUM (sets has_written on all elements).
        # cs PSUM is [32, 512]: row q accumulates local cols [512q, 512q+512)
        # via one-hot stationary operands (PE outputs must start at
        # partition 0/32/64, so rows are selected by the one-hot instead).
        cs_ps = pscs.tile([32, 512], f32, tag="cs_ps")
        nc.tensor.matmul(
            cs_ps[:, :],
            zeros_bf[:],
            znt[:, 0:512],
            start=True,
            stop=False,
            skip_group_check=True,
        )

        # ---- main: work-list order (sorted by last znT column needed) ----
        piece_ctr = [0]
        emitted = {}

        def emit_cs(rp, e_list, upto):
            for c0, w, k, eoff in _cs_pieces(rp):
                if c0 < emitted.get(rp, 0):
                    continue
                if c0 + w > rp * 128 + upto:
                    break
                emitted[rp] = c0 + w
                qq = c0 // 512
                piece_ctr[0] += 1
                nc.tensor.matmul(
                    cs_ps[:, c0 - qq * 512 : c0 - qq * 512 + w],
                    onehot_bf[:, 10 - qq : 42 - qq],
                    e_list[k][:, eoff : eoff + w],
                    start=False,
                    stop=(piece_ctr[0] == _N_PIECES),
                    skip_group_check=True,
                )

        e_tiles = [[None] * len(_chunks(rp)) for rp in range(RT)]
        for rp, k in _WORK:
            lhsT = znt[:, rp * 128 : (rp + 1) * 128]
            base = rp * 128
            w_k = _chunks(rp)[k]
            s_k = _chunk_starts(rp)[k]
            last_k = k == len(_chunks(rp)) - 1
            G_ps = psg.tile([128, 1536], f32, tag="ps_g")
            c0 = base + s_k
            for off in range(0, w_k, 512):
                w = min(512, w_k - off)
                nc.tensor.matmul(
                    G_ps[:, off : off + w],
                    lhsT,
                    znt[:, c0 + off : c0 + off + w],
                    start=True,
                    stop=True,
                )
            e_sb = epool.tile([128, 1536], bf16, tag="e_sb")
            e_tiles[rp][k] = e_sb
            nc.scalar.activation(
                out=e_sb[:, 0:w_k],
                in_=G_ps[:, 0:w_k],
                func=FN.Exp,
                scale=2.0,
                accum_out=accs[:, rp, k : k + 1],
            )
            # diag/pos come from e_sb (SBUF, bf16): masked accum picks
            # e^{2K[a,a]} / e^{2K[a,p(a)]}; host takes log.  Reading e_sb
            # instead of G_ps keeps DVE off the PSUM critical path.
            if k == 0:
                nc.vector.scalar_tensor_tensor(
                    out=m_scr[:],
                    in0=e_sb[:, 0:128],
                    scalar=1.0,
                    in1=ident_bf[:],
                    op0=ALU.mult,
                    op1=ALU.mult,
                    accum_out=misc_sb[:, RT + rp : RT + rp + 1],
                )
            if last_k:
                nc.vector.scalar_tensor_tensor(
                    out=m_scr[:],
                    in0=e_sb[:, 1024:1152],
                    scalar=1.0,
                    in1=ident_bf[:],
                    op0=ALU.mult,
                    op1=ALU.mult,
                    accum_out=misc_sb[:, 2 * RT + rp : 2 * RT + rp + 1],
                )
            emit_cs(rp, e_tiles[rp], s_k + w_k)

        # ---- epilogue ----
        nc.vector.reduce_sum(
            out=misc_sb[:, 0:RT], in_=accs[:], axis=mybir.AxisListType.X
        )
        nc.scalar.copy(out=cs_sb[:], in_=cs_ps[0:10, :])
        nc.sync.dma_start(misc_out.ap()[:, :], misc_sb[:])
        nc.scalar.dma_start(cs_out.ap()[:, :], cs_sb[:])

    # Pin every ACT function to the one table set containing Ln/Exp so the
    # table-load pass never inserts mid-kernel switches (~2.7us each).
    import concourse.hw_specs as hw_specs

    _real_tables = hw_specs.get_activation_tables(nc.m.arch)
    _pruned = {
        name: (fns if name == "natural_log_exp_and_others" else set())
        for name, fns in _real_tables.items()
    }
    _orig_get_tables = bacc.get_activation_tables

    with tile.TileContext(nc) as tc, ExitStack() as ctx:
        const_pool = ctx.enter_context(tc.tile_pool(name="const", bufs=1))
        big = ctx.enter_context(tc.tile_pool(name="big", bufs=1))
        small = ctx.enter_context(tc.tile_pool(name="small", bufs=1))
        scr = ctx.enter_context(tc.tile_pool(name="scr", bufs=1))
        epool = ctx.enter_context(tc.tile_pool(name="epool", bufs=28))
        psg = ctx.enter_context(tc.tile_pool(name="psg", bufs=2, space="PSUM"))
        pscs = ctx.enter_context(tc.tile_pool(name="pscs", bufs=1, space="PSUM"))
        pstr = ctx.enter_context(tc.tile_pool(name="pstr", bufs=1, space="PSUM"))

        import concourse.mybir as mybir

        ident_bf = const_pool.tile([128, 128], mybir.dt.bfloat16, tag="idbf")
        make_identity(nc, ident_bf[:])
        # onehot_bf[:, 10] == 1, else 0; slices [10-q : 42-q] give [128, 32]
        # one-hot stationaries selecting cs PSUM row q.
        onehot_bf = const_pool.tile([128, 42], mybir.dt.bfloat16, tag="onehot")
        nc.vector.memset(onehot_bf[:], 0.0)
        nc.vector.memset(onehot_bf[:, 10:11], 1.0)
        zeros_bf = const_pool.tile([128, 32], mybir.dt.bfloat16, tag="zeros")
        nc.vector.memset(zeros_bf[:], 0.0)

        for _rep in range(reps):
            body(
                (big, small, scr, epool, psg, pscs, pstr),
                (ident_bf, onehot_bf, zeros_bf),
            )

    bacc.get_activation_tables = lambda arch: _pruned
    try:
        nc.compile()
    finally:
        bacc.get_activation_tables = _orig_get_tables
    return nc


def get_nc(reps: int = 1):
    if reps not in _NC_CACHE:
        _NC_CACHE[reps] = _build_nc(reps)
    return _NC_CACHE[reps]


def make_in_maps(z_i: np.ndarray, z_j: np.ndarray):
    z = np.concatenate(
        [np.asarray(z_i, np.float32), np.asarray(z_j, np.float32)], axis=0
    )
    in_maps = []
    for c in range(NCORES):
        zr = np.roll(z, -c * RPC, axis=0)[:LROWS]
        in_maps.append({"z_nat": np.ascontiguousarray(zr)})
    return in_maps


def gather(results) -> np.ndarray:
    S = np.zeros(N, np.float64)
    diag2 = np.zeros(N, np.float64)
    pos2 = np.zeros(N, np.float64)
    for c, res in enumerate(results):
        misc = res["misc"].astype(np.float64)  # [128, 24]
        cs = res["cs"].astype(np.float64).reshape(-1)  # local cols 0..5119
        rows0 = c * RPC
        for rp in range(RT):
            rr = slice(rows0 + rp * 128, rows0 + (rp + 1) * 128)
            S[rr] += misc[:, rp]
            diag2[rr] = np.log(misc[:, RT + rp])
            pos2[rr] = np.log(misc[:, 2 * RT + rp])
        idx = (rows0 + np.arange(cs.shape[0])) % N
        np.add.at(S, idx, cs)
    loss = (np.log(S - np.exp(pos2)) - diag2).sum() / N
    return np.float32(loss)


def kernel(z_i: np.ndarray, z_j: np.ndarray, **run_kwargs) -> np.ndarray:
    from concourse.bass_utils import run_bass_kernel_spmd

    nc = get_nc()
    in_maps = make_in_maps(z_i, z_j)
    res = run_bass_kernel_spmd(
        nc, in_maps, core_ids=list(range(NCORES)), **run_kwargs
    )
    out = gather(res.results)
    kernel.last_results = res
    return out
